# revision 45
# baseline (speedup 1.0000x reference)
"""Multi-head 3D attention (8 heads, C=512, N=16^3=4096) on 8 Trainium2 cores.

Sharding: one head per NeuronCore (head-parallel). Each core receives the
full token activations plus its head's slice of the qkv/out projection
weights, computes its head's attention and its partial contribution to the
output projection; the host sums the 8 partial outputs.

Per-core algorithm (S^T orientation -> no transposes anywhere):
  xT   = x.reshape(C, N)                   # [512, 4096] fp16, channel-major
  qT   = Wq @ xT, kT = Wk @ xT             # [64, 4096] fp16 (dh on partitions)
  v    = xT.T @ Wv.T                       # [4096, 64] bf16 (keys on partitions)
  S^T  = kT-tile.T @ qT                    # [128 keys, 1024 q] PSUM fp32
  P^T  = exp(8 * S^T)                      # one ACT op, scale fused, bf16 out
  o_aug= [v, 1].T @ P^T                    # [65, 1024] PSUM; row 64 = denom
  o    = o_aug[:64] * (1/denom)            # normalized in place, off-path
  outp = w_out_h @ o                       # [512, 4096] fp32 partial

Precision: fp16 (11-bit mantissa) for q/k keeps logit error ~1e-2 absolute
(logits reach +-80 and the softmax is very peaked, so bf16 there is NOT
ok). P must be bf16: the unnormalized exp reaches e^75, which overflows
fp16's range. No softmax max-subtraction: the HW exp is accurate over the
whole fp32 range (verified ~1e-5 rel err) and e^75 fits fp32/bf16 range.
End-to-end absmax relative error vs the fp32 reference: ~4.4e-3.

Performance notes (measured on HW):
 - 2-byte matmul operands stream at 1 cycle/row (213 ns per N=512 matmul
   warm); 4-byte fp32/f32r only manage 2 cycles/row. A matmul's output
   cannot cross a PSUM bank -> N <= 512 per matmul.
 - Steady state is ACT-bound (exp of [128,1024] = 1.11 us per key tile vs
   0.85 us of PE work), so left alone the PE idles in small gaps and the
   HAM clock gate drops it to 1.2 GHz, making cold PE the bottleneck.
   Countermeasures: P^T buffered in SBUF with P @ v trailing one key tile
   behind (o-matmul inputs always ready), dependency-free filler matmuls
   to keep the PE duty cycle high (kept alive by a *0 fold into one output
   tile), and the q/k/v projections emitted just-in-time inside query
   group 0 so they act as real filler during ramp-up instead of a serial
   phase.
"""

import sys

for _p in ("/opt/trn_rl_repo",):
    if _p not in sys.path:
        sys.path.insert(0, _p)

import numpy as np

C = 512          # channels
N = 4096         # tokens (16*16*16)
HEADS = 8
DH = C // HEADS  # 64
SCALE = float(DH) ** 0.5  # 8.0 (reference multiplies by sqrt(dh))
NCORES = 8

KT = 128                 # key-tile size (S^T partition dim)
NKT = N // KT            # 32
QG = 1024                # queries per o-psum accumulation group
NQG = N // QG            # 4
SW = 1024                # S-tile width (queries per exp call)
MV = 512                 # max matmul free dim (one PSUM bank)

_compiled = None


def _build():
    import concourse.tile as tile
    from concourse import bacc, mybir

    F32 = mybir.dt.float32
    F16 = mybir.dt.float16
    BF16 = mybir.dt.bfloat16
    EXP = mybir.ActivationFunctionType.Exp
    NCT = C // 128  # 4 channel tiles

    nc = bacc.Bacc("TRN2", num_devices=NCORES)
    xT_d = nc.dram_tensor("xT", [C, N], F16, kind="ExternalInput")
    # columns 0:64 = Wq^T, 64:128 = Wk^T, 128:192 = Wv^T (this head's rows)
    wqkvT_d = nc.dram_tensor("wqkvT", [C, 3 * DH], F16, kind="ExternalInput")
    # w_out[:, head_cols].T  -> [64, 512]
    w_outT_d = nc.dram_tensor("w_outT", [DH, C], BF16, kind="ExternalInput")
    outp_d = nc.dram_tensor("outp", [C, N], F32, kind="ExternalOutput")

    with tile.TileContext(nc) as tc:
        with tc.tile_pool(name="const", bufs=1) as const:
            # ---- persistent SBUF tensors ----
            xt = [const.tile([128, N], F16, tag=f"x{i}", name=f"x{i}")
                  for i in range(NCT)]
            wqkv = [const.tile([128, 3 * DH], F16, tag=f"w{i}", name=f"w{i}")
                    for i in range(NCT)]
            woutT = const.tile([DH, C], BF16, tag="wo")
            qT = const.tile([DH, N], F16, tag="qT")
            kT = const.tile([DH, N], F16, tag="kT")
            vaug = const.tile([128, NKT, DH + 1], BF16, tag="vaug")
            o_sb = const.tile([DH, N], BF16, tag="o")        # o^T
            den = const.tile([1, N], F32, tag="den")         # softmax denom
            recip = const.tile([1, N], F32, tag="recip")     # 1/denominator
            recipb = const.tile([DH, N], F32, tag="recipb")  # bcast to 64p
            # P^T tiles for one full query group (decouples P@v from ACT)
            pstore = const.tile([128, NKT, SW], BF16, tag="pstore")
            ones = const.tile([128, 1], F32, tag="ones")
            nc.vector.memset(ones, 1.0)

            # weights first, then the token chunks needed soonest; spread
            # across engine DMA queues so the first chunks land in parallel
            engines = [nc.sync, nc.sync, nc.sync, nc.sync]
            for i in range(NCT):
                engines[i % 4].dma_start(
                    out=wqkv[i], in_=wqkvT_d.ap()[i * 128:(i + 1) * 128, :])
            nc.sync.dma_start(out=woutT, in_=w_outT_d.ap())
            qn = 0
            for lo, hi in ((0, 512), (512, 1024), (1024, 2048), (2048, N)):
                for i in range(NCT):
                    engines[qn % 4].dma_start(
                        out=xt[i][:, lo:hi],
                        in_=xT_d.ap()[i * 128:(i + 1) * 128, lo:hi])
                    qn += 1

            def qk_chunk(pool, ch):
                """q/k projection for token chunk ch -> qT/kT[:, ch*512:...].
                One [128, 512] psum tile: q rows on partitions 0:64, k rows
                on partitions 64:128."""
                sl = slice(ch * MV, (ch + 1) * MV)
                ps = pool.tile([128, MV], F32, tag="jit", name=f"psqk{ch}")
                for ct in range(NCT):
                    nc.tensor.matmul(ps[0:DH, :], lhsT=wqkv[ct][:, 0:DH],
                                     rhs=xt[ct][:, sl],
                                     start=(ct == 0), stop=(ct == NCT - 1))
                for ct in range(NCT):
                    nc.tensor.matmul(ps[DH:2 * DH, :],
                                     lhsT=wqkv[ct][:, DH:2 * DH],
                                     rhs=xt[ct][:, sl],
                                     start=(ct == 0), stop=(ct == NCT - 1))
                nc.vector.tensor_copy(out=qT[:, sl], in_=ps[0:DH, :])
                nc.vector.tensor_copy(out=kT[:, sl], in_=ps[DH:2 * DH, :])

            def k_chunk(pool, ch):
                """k-only projection for token chunk ch (qT deferred)."""
                sl = slice(ch * MV, (ch + 1) * MV)
                ps = pool.tile([128, MV], F32, tag="jit", name=f"psk{ch}")
                for ct in range(NCT):
                    nc.tensor.matmul(ps[DH:2 * DH, :],
                                     lhsT=wqkv[ct][:, DH:2 * DH],
                                     rhs=xt[ct][:, sl],
                                     start=(ct == 0), stop=(ct == NCT - 1))
                nc.vector.tensor_copy(out=kT[:, sl], in_=ps[DH:2 * DH, :])

            def q_chunk(pool, ch):
                """deferred q-only projection (emitted in a later query
                group's filler slots; reads only constant inputs)."""
                sl = slice(ch * MV, (ch + 1) * MV)
                ps = pool.tile([128, MV], F32, tag="jit", name=f"psq{ch}")
                for ct in range(NCT):
                    nc.tensor.matmul(ps[0:DH, :], lhsT=wqkv[ct][:, 0:DH],
                                     rhs=xt[ct][:, sl],
                                     start=(ct == 0), stop=(ct == NCT - 1))
                nc.vector.tensor_copy(out=qT[:, sl], in_=ps[0:DH, :])

            def v_tile(pool, kt_i):
                """v projection for key tile kt_i -> vaug[:, kt_i, :]."""
                ps = pool.tile([128, MV], F32, tag="jit", name=f"psv{kt_i}")
                for ct in range(NCT):
                    nc.tensor.matmul(ps[:, 0:DH],
                                     lhsT=xt[ct][:, kt_i * KT:(kt_i + 1) * KT],
                                     rhs=wqkv[ct][:, 2 * DH:3 * DH],
                                     start=(ct == 0), stop=(ct == NCT - 1))
                nc.vector.tensor_copy(out=vaug[:, kt_i, 0:DH], in_=ps[:, 0:DH])
                nc.vector.tensor_copy(out=vaug[:, kt_i, DH:DH + 1], in_=ones)

            with tc.tile_pool(name="misc", bufs=2, space="PSUM") as misc:
                # warm-up fillers: only need the (tiny, fast) weight DMAs,
                # so they run while the big xT DMA streams in -- the PE
                # starts phase 1 already at 2.4 GHz instead of idling cold
                last_filler = None
                for wf in range(16):
                    last_filler = misc.tile([128, MV], F32, tag="jit",
                                            name=f"warm{wf}")
                    nc.tensor.matmul(last_filler[:, 0:192],
                                     lhsT=wqkv[wf % NCT][:, 0:128],
                                     rhs=wqkv[(wf + 1) % NCT][:, :],
                                     start=True, stop=True,
                                     skip_group_check=True)

                # ---- phase 1: first two q/k chunks (needed by the very
                # ---- first S matmuls); the rest is emitted just-in-time
                with tc.tile_pool(name="ph1", bufs=2, space="PSUM") as ph1:
                    qk_chunk(ph1, 0)
                    qk_chunk(ph1, 1)

                # ---- phase 2: attention ----
                s_ps_cm = tc.tile_pool(name="s_ps", bufs=2, space="PSUM")
                o_ps_cm = tc.tile_pool(name="o_ps", bufs=1, space="PSUM")
                s_ps = s_ps_cm.__enter__()
                o_ps = o_ps_cm.__enter__()
                for qg in range(NQG):
                    q0 = qg * QG
                    ops = o_ps.tile([DH + 1, QG], F32, tag="ops",
                                    name=f"ops{qg}")
                    for kt_i in range(NKT + 1):
                        if qg == 0 and kt_i < NKT:
                            # just-in-time projections double as PE filler;
                            # q-halves of chunks 4-7 are deferred to the
                            # filler slots of qg1/qg2 (not needed earlier)
                            if kt_i % 4 == 0 and kt_i // 4 + 2 < N // MV:
                                c = kt_i // 4 + 2
                                (qk_chunk if c < 4 else k_chunk)(misc, c)
                            v_tile(misc, kt_i)
                        if qg in (1, 2) and kt_i in (12, 16):
                            q_chunk(misc, 2 + 2 * qg + (kt_i - 12) // 4)
                        if kt_i < NKT:
                            sps = s_ps.tile([128, SW], F32, tag="s",
                                            name=f"sps{qg}_{kt_i}")
                            for mv in range(SW // MV):
                                nc.tensor.matmul(
                                    sps[:, mv * MV:(mv + 1) * MV],
                                    lhsT=kT[:, kt_i * KT:(kt_i + 1) * KT],
                                    rhs=qT[:, q0 + mv * MV: q0 + (mv + 1) * MV],
                                    start=True, stop=True)
                            nc.scalar.activation(out=pstore[:, kt_i, :],
                                                 in_=sps, func=EXP, scale=SCALE)
                        if kt_i >= 1:
                            ot_i = kt_i - 1
                            for mv in range(SW // MV):
                                nc.tensor.matmul(
                                    ops[:, mv * MV:(mv + 1) * MV],
                                    lhsT=vaug[:, ot_i, :],
                                    rhs=pstore[:, ot_i, mv * MV:(mv + 1) * MV],
                                    start=(ot_i == 0),
                                    stop=(ot_i == NKT - 1))
                        if qg >= 1 and not (qg in (1, 2) and kt_i in (12, 16)):
                            # HAM-warming filler (dependency-free)
                            last_filler = misc.tile([128, MV], F32, tag="jit",
                                                    name=f"fill{qg}_{kt_i}")
                            nc.tensor.matmul(last_filler[:, 0:320],
                                             lhsT=kT[:, 0:KT],
                                             rhs=qT[:, 0:320], start=True,
                                             stop=True, skip_group_check=True)
                    # fast flush (frees the o psum tile quickly); the
                    # normalization chain runs off the critical path and
                    # normalizes o_sb in place before phase 3 reads it
                    nc.vector.tensor_copy(out=o_sb[:, q0:q0 + QG],
                                          in_=ops[0:DH, :])
                    nc.scalar.copy(out=den[:, q0:q0 + QG],
                                   in_=ops[DH:DH + 1, :])
                    for hh in range(QG // MV):
                        hsl = slice(q0 + hh * MV, q0 + (hh + 1) * MV)
                        nc.vector.reciprocal(out=recip[:, hsl],
                                             in_=den[:, hsl])
                        nc.gpsimd.partition_broadcast(recipb[:, hsl],
                                                      recip[:, hsl])
                        nc.vector.tensor_mul(o_sb[:, hsl], o_sb[:, hsl],
                                             recipb[:, hsl])

                # ---- phase 3: output projection ----
                with tc.tile_pool(name="out_ps", bufs=3, space="PSUM") as out_ps, \
                     tc.tile_pool(name="out_sb", bufs=4) as out_sb:
                    for ch in range(N // 1024):
                        for ct in range(NCT):
                            sl = slice(ch * 1024, (ch + 1) * 1024)
                            pso = out_ps.tile([128, 1024], F32, tag="pso",
                                              name=f"pso{ch}_{ct}")
                            for mv in range(2):
                                msl = slice(ch * 1024 + mv * MV,
                                            ch * 1024 + (mv + 1) * MV)
                                nc.tensor.matmul(
                                    pso[:, mv * MV:(mv + 1) * MV],
                                    lhsT=woutT[:, ct * 128:(ct + 1) * 128],
                                    rhs=o_sb[:, msl], start=True, stop=True)
                            ot = out_sb.tile([128, 1024], F32, tag="ot",
                                             name=f"ot{ch}_{ct}")
                            if (ch * NCT + ct) % 2 == 0:
                                nc.scalar.copy(out=ot, in_=pso)
                            else:
                                nc.vector.tensor_copy(out=ot, in_=pso)
                            if ch == 0 and ct == 0:
                                # + 0 * scratch keeps the fillers alive
                                nc.vector.scalar_tensor_tensor(
                                    out=ot[:, 0:MV], in0=last_filler,
                                    scalar=0.0, in1=ot[:, 0:MV],
                                    op0=mybir.AluOpType.mult,
                                    op1=mybir.AluOpType.add)
                            nc.sync.dma_start(
                                out=outp_d.ap()[ct * 128:(ct + 1) * 128, sl],
                                in_=ot)

    nc.compile()
    return nc


def _get_compiled():
    global _compiled
    if _compiled is None:
        _compiled = _build()
    return _compiled


def make_in_maps(x, w_qkv, w_out):
    import ml_dtypes
    xT = np.ascontiguousarray(x.reshape(C, N).astype(np.float16))
    in_maps = []
    for h in range(NCORES):
        rows = np.concatenate([
            np.arange(h * DH, (h + 1) * DH),
            np.arange(C + h * DH, C + (h + 1) * DH),
            np.arange(2 * C + h * DH, 2 * C + (h + 1) * DH),
        ])
        wqkvT = np.ascontiguousarray(w_qkv[rows, :].T.astype(np.float16))
        w_outT = np.ascontiguousarray(
            w_out[:, h * DH:(h + 1) * DH].T.astype(ml_dtypes.bfloat16))
        in_maps.append({"xT": xT, "wqkvT": wqkvT, "w_outT": w_outT})
    return in_maps


def kernel(x, w_qkv, w_out):
    from concourse.bass_utils import run_bass_kernel_spmd

    x = np.ascontiguousarray(np.asarray(x), dtype=np.float32)
    w_qkv = np.ascontiguousarray(np.asarray(w_qkv), dtype=np.float32)
    w_out = np.ascontiguousarray(np.asarray(w_out), dtype=np.float32)

    nc = _get_compiled()
    res = run_bass_kernel_spmd(nc, make_in_maps(x, w_qkv, w_out),
                               core_ids=list(range(NCORES)))

    out = np.zeros((C, N), dtype=np.float32)
    for r in res.results:
        out += r["outp"]
    return out.reshape(1, C, 16, 16, 16)


# revision 46
# speedup vs baseline: 1.1276x; 1.1276x over previous
"""Multi-head 3D attention (8 heads, C=512, N=16^3=4096) on 8 Trainium2 cores.

Sharding: one head per NeuronCore (head-parallel). Each core receives the
full token activations plus its head's slice of the qkv/out projection
weights, computes its head's attention and its partial contribution to the
output projection; the host sums the 8 partial outputs.

Per-core algorithm (S^T orientation -> no transposes anywhere):
  xT   = x.reshape(C, N)                   # [512, 4096] fp16, channel-major
  qT   = Wq @ xT, kT = Wk @ xT             # [64, 4096] fp16 (dh on partitions)
  v    = xT.T @ Wv.T                       # [4096, 64] bf16 (keys on partitions)
  S^T  = kT-tile.T @ qT                    # [128 keys, 1024 q] PSUM fp32
  P^T  = exp(8 * S^T)                      # one ACT op, scale fused, bf16 out
  o_aug= [v, 1].T @ P^T                    # [65, 1024] PSUM; row 64 = denom
  o    = o_aug[:64] * (1/denom)            # normalized in place, off-path
  outp = w_out_h @ o                       # [512, 4096] fp32 partial

Precision: fp16 (11-bit mantissa) for q/k keeps logit error ~1e-2 absolute
(logits reach +-80 and the softmax is very peaked, so bf16 there is NOT
ok). P must be bf16: the unnormalized exp reaches e^75, which overflows
fp16's range. No softmax max-subtraction: the HW exp is accurate over the
whole fp32 range (verified ~1e-5 rel err) and e^75 fits fp32/bf16 range.
End-to-end absmax relative error vs the fp32 reference: ~4.4e-3.

Performance notes (measured on HW):
 - 2-byte matmul operands stream at 1 cycle/row (213 ns per N=512 matmul
   warm); 4-byte fp32/f32r only manage 2 cycles/row. A matmul's output
   cannot cross a PSUM bank -> N <= 512 per matmul.
 - Steady state is ACT-bound (exp of [128,1024] = 1.11 us per key tile vs
   0.85 us of PE work), so left alone the PE idles in small gaps and the
   HAM clock gate drops it to 1.2 GHz, making cold PE the bottleneck.
   Countermeasures: P^T buffered in SBUF with P @ v trailing one key tile
   behind (o-matmul inputs always ready), dependency-free filler matmuls
   to keep the PE duty cycle high (kept alive by a *0 fold into one output
   tile), and the q/k/v projections emitted just-in-time inside query
   group 0 so they act as real filler during ramp-up instead of a serial
   phase.
"""

import sys

for _p in ("/opt/trn_rl_repo",):
    if _p not in sys.path:
        sys.path.insert(0, _p)

import numpy as np

C = 512          # channels
N = 4096         # tokens (16*16*16)
HEADS = 8
DH = C // HEADS  # 64
SCALE = float(DH) ** 0.5  # 8.0 (reference multiplies by sqrt(dh))
NCORES = 8

KT = 128                 # key-tile size (S^T partition dim)
NKT = N // KT            # 32
QG = 1024                # queries per o-psum accumulation group
NQG = N // QG            # 4
SW = 1024                # S-tile width (queries per exp call)
MV = 512                 # max matmul free dim (one PSUM bank)

_compiled = None


def _build():
    import concourse.tile as tile
    from concourse import bacc, mybir

    F32 = mybir.dt.float32
    F16 = mybir.dt.float16
    BF16 = mybir.dt.bfloat16
    EXP = mybir.ActivationFunctionType.Exp
    NCT = C // 128  # 4 channel tiles

    nc = bacc.Bacc("TRN2", num_devices=NCORES)
    xT_d = nc.dram_tensor("xT", [C, N], F16, kind="ExternalInput")
    # columns 0:64 = Wq^T, 64:128 = Wk^T, 128:192 = Wv^T (this head's rows)
    wqkvT_d = nc.dram_tensor("wqkvT", [C, 3 * DH], F16, kind="ExternalInput")
    # w_out[:, head_cols].T  -> [64, 512]
    w_outT_d = nc.dram_tensor("w_outT", [DH, C], BF16, kind="ExternalInput")
    outp_d = nc.dram_tensor("outp", [C, N], F32, kind="ExternalOutput")

    with tile.TileContext(nc) as tc:
        with tc.tile_pool(name="const", bufs=1) as const:
            # ---- persistent SBUF tensors ----
            xt = [const.tile([128, N], F16, tag=f"x{i}", name=f"x{i}")
                  for i in range(NCT)]
            wqkv = [const.tile([128, 3 * DH], F16, tag=f"w{i}", name=f"w{i}")
                    for i in range(NCT)]
            woutT = const.tile([DH, C], BF16, tag="wo")
            qT = const.tile([DH, N], F16, tag="qT")
            kT = const.tile([DH, N], F16, tag="kT")
            vaug = const.tile([128, NKT, DH + 1], BF16, tag="vaug")
            o_sb = const.tile([DH, N], BF16, tag="o")        # o^T
            den = const.tile([1, N], F32, tag="den")         # softmax denom
            recip = const.tile([1, N], F32, tag="recip")     # 1/denominator
            recipb = const.tile([DH, N], F32, tag="recipb")  # bcast to 64p
            # P^T tiles for one full query group (decouples P@v from ACT)
            pstore = const.tile([128, NKT, SW], BF16, tag="pstore")
            ones = const.tile([128, 1], F32, tag="ones")
            nc.vector.memset(ones, 1.0)

            # weights first, then the token chunks needed soonest; spread
            # across engine DMA queues so the first chunks land in parallel
            engines = [nc.sync, nc.sync, nc.sync, nc.sync]
            for i in range(NCT):
                engines[i % 4].dma_start(
                    out=wqkv[i], in_=wqkvT_d.ap()[i * 128:(i + 1) * 128, :])
            nc.sync.dma_start(out=woutT, in_=w_outT_d.ap())
            qn = 0
            for lo, hi in ((0, 512), (512, 1024), (1024, 2048), (2048, N)):
                for i in range(NCT):
                    engines[qn % 4].dma_start(
                        out=xt[i][:, lo:hi],
                        in_=xT_d.ap()[i * 128:(i + 1) * 128, lo:hi])
                    qn += 1

            def qk_chunk(pool, ch):
                """q/k projection for token chunk ch -> qT/kT[:, ch*512:...].
                One [128, 512] psum tile: q rows on partitions 0:64, k rows
                on partitions 64:128."""
                sl = slice(ch * MV, (ch + 1) * MV)
                ps = pool.tile([128, MV], F32, tag="jit", name=f"psqk{ch}")
                for ct in range(NCT):
                    nc.tensor.matmul(ps[0:DH, :], lhsT=wqkv[ct][:, 0:DH],
                                     rhs=xt[ct][:, sl],
                                     start=(ct == 0), stop=(ct == NCT - 1))
                for ct in range(NCT):
                    nc.tensor.matmul(ps[DH:2 * DH, :],
                                     lhsT=wqkv[ct][:, DH:2 * DH],
                                     rhs=xt[ct][:, sl],
                                     start=(ct == 0), stop=(ct == NCT - 1))
                nc.vector.tensor_copy(out=qT[:, sl], in_=ps[0:DH, :])
                nc.vector.tensor_copy(out=kT[:, sl], in_=ps[DH:2 * DH, :])

            def v_tile(pool, kt_i):
                """v projection for key tile kt_i -> vaug[:, kt_i, :]."""
                ps = pool.tile([128, MV], F32, tag="jit", name=f"psv{kt_i}")
                for ct in range(NCT):
                    nc.tensor.matmul(ps[:, 0:DH],
                                     lhsT=xt[ct][:, kt_i * KT:(kt_i + 1) * KT],
                                     rhs=wqkv[ct][:, 2 * DH:3 * DH],
                                     start=(ct == 0), stop=(ct == NCT - 1))
                nc.vector.tensor_copy(out=vaug[:, kt_i, 0:DH], in_=ps[:, 0:DH])
                nc.vector.tensor_copy(out=vaug[:, kt_i, DH:DH + 1], in_=ones)

            with tc.tile_pool(name="misc", bufs=2, space="PSUM") as misc:
                # warm-up fillers: only need the (tiny, fast) weight DMAs,
                # so they run while the big xT DMA streams in -- the PE
                # starts phase 1 already at 2.4 GHz instead of idling cold
                last_filler = None
                for wf in range(16):
                    last_filler = misc.tile([128, MV], F32, tag="jit",
                                            name=f"warm{wf}")
                    nc.tensor.matmul(last_filler[:, 0:192],
                                     lhsT=wqkv[wf % NCT][:, 0:128],
                                     rhs=wqkv[(wf + 1) % NCT][:, :],
                                     start=True, stop=True,
                                     skip_group_check=True)

                # ---- phase 1: first two q/k chunks (needed by the very
                # ---- first S matmuls); the rest is emitted just-in-time
                with tc.tile_pool(name="ph1", bufs=2, space="PSUM") as ph1:
                    qk_chunk(ph1, 0)
                    qk_chunk(ph1, 1)

                # ---- phase 2: attention ----
                s_ps_cm = tc.tile_pool(name="s_ps", bufs=2, space="PSUM")
                o_ps_cm = tc.tile_pool(name="o_ps", bufs=1, space="PSUM")
                s_ps = s_ps_cm.__enter__()
                o_ps = o_ps_cm.__enter__()
                for qg in range(NQG):
                    q0 = qg * QG
                    ops = o_ps.tile([DH + 1, QG], F32, tag="ops",
                                    name=f"ops{qg}")
                    for kt_i in range(NKT + 1):
                        if qg == 0 and kt_i < NKT:
                            # just-in-time projections double as PE filler
                            if kt_i % 4 == 0 and kt_i // 4 + 2 < N // MV:
                                qk_chunk(misc, kt_i // 4 + 2)
                            v_tile(misc, kt_i)
                        if kt_i < NKT:
                            sps = s_ps.tile([128, SW], F32, tag="s",
                                            name=f"sps{qg}_{kt_i}")
                            for mv in range(SW // MV):
                                nc.tensor.matmul(
                                    sps[:, mv * MV:(mv + 1) * MV],
                                    lhsT=kT[:, kt_i * KT:(kt_i + 1) * KT],
                                    rhs=qT[:, q0 + mv * MV: q0 + (mv + 1) * MV],
                                    start=True, stop=True)
                            nc.scalar.activation(out=pstore[:, kt_i, :],
                                                 in_=sps, func=EXP, scale=SCALE)
                        if kt_i >= 1:
                            ot_i = kt_i - 1
                            for mv in range(SW // MV):
                                nc.tensor.matmul(
                                    ops[:, mv * MV:(mv + 1) * MV],
                                    lhsT=vaug[:, ot_i, :],
                                    rhs=pstore[:, ot_i, mv * MV:(mv + 1) * MV],
                                    start=(ot_i == 0),
                                    stop=(ot_i == NKT - 1))
                        if qg >= 1:
                            # HAM-warming filler (dependency-free)
                            last_filler = misc.tile([128, MV], F32, tag="jit",
                                                    name=f"fill{qg}_{kt_i}")
                            nc.tensor.matmul(last_filler[:, 0:320],
                                             lhsT=kT[:, 0:KT],
                                             rhs=qT[:, 0:320], start=True,
                                             stop=True, skip_group_check=True)
                    # fast flush (frees the o psum tile quickly); the
                    # normalization chain runs off the critical path and
                    # normalizes o_sb in place before phase 3 reads it
                    nc.vector.tensor_copy(out=o_sb[:, q0:q0 + QG],
                                          in_=ops[0:DH, :])
                    nc.scalar.copy(out=den[:, q0:q0 + QG],
                                   in_=ops[DH:DH + 1, :])
                    for hh in range(QG // MV):
                        hsl = slice(q0 + hh * MV, q0 + (hh + 1) * MV)
                        nc.vector.reciprocal(out=recip[:, hsl],
                                             in_=den[:, hsl])
                        nc.gpsimd.partition_broadcast(recipb[:, hsl],
                                                      recip[:, hsl])
                        nc.vector.tensor_mul(o_sb[:, hsl], o_sb[:, hsl],
                                             recipb[:, hsl])

                # ---- phase 3: output projection ----
                with tc.tile_pool(name="out_ps", bufs=3, space="PSUM") as out_ps, \
                     tc.tile_pool(name="out_sb", bufs=4) as out_sb:
                    for ch in range(N // 1024):
                        for ct in range(NCT):
                            sl = slice(ch * 1024, (ch + 1) * 1024)
                            pso = out_ps.tile([128, 1024], F32, tag="pso",
                                              name=f"pso{ch}_{ct}")
                            for mv in range(2):
                                msl = slice(ch * 1024 + mv * MV,
                                            ch * 1024 + (mv + 1) * MV)
                                nc.tensor.matmul(
                                    pso[:, mv * MV:(mv + 1) * MV],
                                    lhsT=woutT[:, ct * 128:(ct + 1) * 128],
                                    rhs=o_sb[:, msl], start=True, stop=True)
                            ot = out_sb.tile([128, 1024], F32, tag="ot",
                                             name=f"ot{ch}_{ct}")
                            if (ch * NCT + ct) % 2 == 0:
                                nc.scalar.copy(out=ot, in_=pso)
                            else:
                                nc.vector.tensor_copy(out=ot, in_=pso)
                            if ch == 0 and ct == 0:
                                # + 0 * scratch keeps the fillers alive
                                nc.vector.scalar_tensor_tensor(
                                    out=ot[:, 0:MV], in0=last_filler,
                                    scalar=0.0, in1=ot[:, 0:MV],
                                    op0=mybir.AluOpType.mult,
                                    op1=mybir.AluOpType.add)
                            nc.sync.dma_start(
                                out=outp_d.ap()[ct * 128:(ct + 1) * 128, sl],
                                in_=ot)

    nc.compile()
    return nc


def _get_compiled():
    global _compiled
    if _compiled is None:
        _compiled = _build()
    return _compiled


def make_in_maps(x, w_qkv, w_out):
    import ml_dtypes
    xT = np.ascontiguousarray(x.reshape(C, N).astype(np.float16))
    in_maps = []
    for h in range(NCORES):
        rows = np.concatenate([
            np.arange(h * DH, (h + 1) * DH),
            np.arange(C + h * DH, C + (h + 1) * DH),
            np.arange(2 * C + h * DH, 2 * C + (h + 1) * DH),
        ])
        wqkvT = np.ascontiguousarray(w_qkv[rows, :].T.astype(np.float16))
        w_outT = np.ascontiguousarray(
            w_out[:, h * DH:(h + 1) * DH].T.astype(ml_dtypes.bfloat16))
        in_maps.append({"xT": xT, "wqkvT": wqkvT, "w_outT": w_outT})
    return in_maps


def kernel(x, w_qkv, w_out):
    from concourse.bass_utils import run_bass_kernel_spmd

    x = np.ascontiguousarray(np.asarray(x), dtype=np.float32)
    w_qkv = np.ascontiguousarray(np.asarray(w_qkv), dtype=np.float32)
    w_out = np.ascontiguousarray(np.asarray(w_out), dtype=np.float32)

    nc = _get_compiled()
    res = run_bass_kernel_spmd(nc, make_in_maps(x, w_qkv, w_out),
                               core_ids=list(range(NCORES)))

    out = np.zeros((C, N), dtype=np.float32)
    for r in res.results:
        out += r["outp"]
    return out.reshape(1, C, 16, 16, 16)
